# revision 20
# baseline (speedup 1.0000x reference)
"""Ragged-sequence multi-head attention (B=16, S=1024, D=512, H=8, DH=64)
for 8 Trainium2 NeuronCores.

Strategy v2: shard by (head-pair x sequence-group). The 16 sequences are
sorted by length and paired (1st+2nd, 3rd+4th, ...); each pair defines one
"slot" whose static k-tile count is the max of the two -> a common slot
profile shared by both groups. Group A takes the first of each pair, group
B the second. Cores = 4 head-pairs x 2 groups; every core runs the SAME
SPMD program over its group's packed sequences, computing Q/K/V, ragged
attention (per-slot loop bounds = that slot's k-tile count), and a
TRANSPOSED partial output projection with its head-pair's Wo slice. The
host sums the 4 per-pair partials of each group and adds bo.

vs the 2-slot baseline this removes ~45% of the attention area (each slot
pays only its own length, not the global max) and ~35% of projection work.

Per-core pipeline:
  1. QT/KT [128=2x64 dims, C] via stationary Wq/Wk pair-slices (4 kc chunks)
  2. V [128 keys, kt, 128 dims] via stationary xT k-tiles, moving Wv slice
  3. per slot, per q-chunk, per k-tile:
       scoresT[k, q] = K^T q (row-packed head pair)
       expT = exp(0.125*scoresT + key_mask_bias)   (ACT)
       outT[d, q] += V^T expT, denom += 1^T expT   (col-packed pairs)
  4. attnT = outT * reciprocal(denom)              (DVE)
  5. partial^T[o, q] = Wo_chunk^T @ attnT (4 persistent 128-col stationaries)
"""

import math
import os

import numpy as np

B, S, D = 16, 1024, 512
H, DH = 8, 64
N_CORES = 8
P = 128
KC = D // P  # 4 contraction chunks of 128
N_PAIRS = 4  # head pairs
N_GROUPS = 2  # sequence groups
N_SLOTS = B // N_GROUPS  # 8 slots per group

_BUILD_CACHE: dict = {}


def _qchunks(nt: int) -> list[int]:
    """Split nt 128-col tiles into balanced chunks of <=4 tiles each."""
    n = -(-nt // 4)
    base, rem = divmod(nt, n)
    return [(base + (1 if i < rem else 0)) * P for i in range(n)]


def _build_bass(profile: tuple[int, ...]):
    """Build the SPMD Bass program for a slot profile (k-tile counts)."""
    from contextlib import ExitStack

    import concourse.bass as bass
    import concourse.mybir as mybir
    import concourse.tile as tile
    from concourse import bacc

    fp32 = mybir.dt.float32
    fp16 = mybir.dt.float16
    Exp = mybir.ActivationFunctionType.Exp
    mult = mybir.AluOpType.mult

    NT = sum(profile)  # total k-tiles
    C = NT * P  # packed columns
    toff = [0] * len(profile)  # slot -> first global k-tile
    for j in range(1, len(profile)):
        toff[j] = toff[j - 1] + profile[j - 1]

    nc = bacc.Bacc("TRN2", target_bir_lowering=False, debug=False)

    xt_d = nc.dram_tensor("xt", [P, KC, C], fp16, kind="ExternalInput").ap()
    w_d = {
        name: nc.dram_tensor(name, [P, KC, P], fp16, kind="ExternalInput").ap()
        for name in ("wq", "wk", "wv", "wo")
    }
    kbias_d = nc.dram_tensor("kbias", [P, NT], fp32, kind="ExternalInput").ap()
    outp_d = nc.dram_tensor("outp", [P, KC, C], fp16, kind="ExternalOutput").ap()

    with ExitStack() as ctx:
        tc = ctx.enter_context(tile.TileContext(nc))
        singles = ctx.enter_context(tc.tile_pool(name="singles", bufs=1))
        big = ctx.enter_context(tc.tile_pool(name="big", bufs=1))
        epool = ctx.enter_context(tc.tile_pool(name="epool", bufs=3))
        opool = ctx.enter_context(tc.tile_pool(name="opool", bufs=4))
        mmps = ctx.enter_context(tc.tile_pool(name="mmps", bufs=2, space="PSUM"))
        scps = ctx.enter_context(tc.tile_pool(name="scps", bufs=2, space="PSUM"))
        accps = ctx.enter_context(tc.tile_pool(name="accps", bufs=1, space="PSUM"))

        ones64 = singles.tile([P, DH], fp16)
        nc.vector.memset(ones64, 1.0)
        warm = singles.tile([P, P], fp16)
        nc.vector.memset(warm, 0.0)

        w_sb = {
            name: singles.tile([P, KC, P], fp16, name=f"w_{name}")
            for name in ("wq", "wk", "wv", "wo")
        }
        kbias_sb = singles.tile([P, NT], fp32)

        # ---- input DMA, all on the sync hw queue, ordered by first use ----
        nc.sync.dma_start(out=w_sb["wq"], in_=w_d["wq"])
        nc.sync.dma_start(out=w_sb["wk"], in_=w_d["wk"])
        xT = big.tile([P, KC, C], fp16, name="xT", tag="xT")
        xchunks = [(cs, min(512, C - cs)) for cs in range(0, C, 512)]

        def xt_dma(idx):
            cs, w = xchunks[idx]
            nc.sync.dma_start(out=xT[:, :, cs : cs + w], in_=xt_d[:, :, cs : cs + w])

        for i in range(min(2, len(xchunks))):
            xt_dma(i)
        nc.sync.dma_start(out=w_sb["wv"], in_=w_d["wv"])
        for i in range(2, min(4, len(xchunks))):
            xt_dma(i)
        nc.sync.dma_start(out=kbias_sb, in_=kbias_d)
        for i in range(4, len(xchunks)):
            xt_dma(i)
        nc.sync.dma_start(out=w_sb["wo"], in_=w_d["wo"])

        # ---- PE/ACT warm-up while input DMA streams (no data deps) ----
        act_dummy = singles.tile([P, 2], fp32)
        nc.scalar.activation(act_dummy[:, 0:1], warm[:, 0:1], Exp, scale=1.0)
        for i in range(16):
            wps = mmps.tile([P, 512], fp32, name="warm_ps", tag="mm")
            nc.tensor.matmul(wps[0:DH, :P], ones64, warm[:, :P], start=True, stop=True)

        QT = big.tile([P, C], fp16, name="QT", tag="QT")
        KT = big.tile([P, C], fp16, name="KT", tag="KT")
        V = big.tile([P, NT, P], fp16, name="V", tag="V")
        attnT = big.tile([P, C], fp16, name="attnT", tag="attnT")

        def qk_units(dst, wname, qs, w):
            """Project x cols [qs, qs+w) with the pair's Wq/Wk slice."""
            ps_box = []

            def mk_mm(kc):
                def emit():
                    if not ps_box:
                        ps_box.append(mmps.tile([P, 512], fp32, name="qk_ps", tag="mm"))
                    nc.tensor.matmul(
                        ps_box[0][:, :w],
                        w_sb[wname][:, kc, :],
                        xT[:, kc, qs : qs + w],
                        start=(kc == 0),
                        stop=(kc == KC - 1),
                    )
                return emit

            def fin():
                # half-partition copies so RAW deps match score matmul reads
                nc.vector.tensor_copy(out=dst[0:DH, qs : qs + w], in_=ps_box[0][0:DH, :w])
                nc.vector.tensor_copy(out=dst[DH:P, qs : qs + w], in_=ps_box[0][DH:P, :w])

            return [mk_mm(kc) for kc in range(KC)] + [fin]

        def v_units(gkt):
            """V tile for global k-tile gkt: [128 keys, 128 pair dims]."""
            ps_box = []

            def mk_mm(kc):
                def emit():
                    if not ps_box:
                        ps_box.append(mmps.tile([P, 512], fp32, name="v_ps", tag="mm"))
                    nc.tensor.matmul(
                        ps_box[0][:, :P],
                        xT[:, kc, gkt * P : (gkt + 1) * P],
                        w_sb["wv"][:, kc, :],
                        start=(kc == 0),
                        stop=(kc == KC - 1),
                    )
                return emit

            def fin():
                nc.vector.tensor_copy(out=V[:, gkt, :], in_=ps_box[0][:, :P])

            return [mk_mm(kc) for kc in range(KC)] + [fin]

        def o_units(chunk_list, final=False):
            """Transposed partial out-proj, oc-major over 1-2 q-chunks so
            each Wo chunk stationary is loaded once per batch. In the final
            drain (no exp work left) casts alternate scalar/vector."""
            units = []
            for oc in range(KC):
                for qs, w in chunk_list:
                    def mk(oc, qs, w):
                        ps_box = []

                        def emit_mm():
                            ps_box.append(
                                mmps.tile([P, 512], fp32, name="o_ps", tag="mm")
                            )
                            nc.tensor.matmul(
                                ps_box[0][:, :w],
                                w_sb["wo"][:, oc, :],
                                attnT[:, qs : qs + w],
                                start=True,
                                stop=True,
                            )

                        def emit_fin():
                            fout = opool.tile([P, 512], fp16, tag="fout")
                            if final and oc % 2 == 1:
                                nc.scalar.copy(out=fout[:, :w], in_=ps_box[0][:, :w])
                            else:
                                nc.vector.tensor_copy(
                                    out=fout[:, :w], in_=ps_box[0][:, :w]
                                )
                            nc.sync.dma_start(
                                out=outp_d[:, oc, qs : qs + w], in_=fout[:, :w]
                            )

                        return [emit_mm, emit_fin]

                    units.extend(mk(oc, qs, w))
            return units

        def attn_chunk(j, qs, w, filler, iters_left):
            """Attention for slot j, q cols [qs, qs+w), both heads of pair."""
            nt = profile[j]
            o_ps = accps.tile([P, 512], fp32, name="opv_ps", tag="opv_ps")
            d_ps = accps.tile([P, 512], fp32, name="d_ps", tag="d_ps")

            def emit_scores_exp(kt):
                gkt = toff[j] + kt
                ks = gkt * P
                s_pair = scps.tile([P, 1024], fp32, name="s_pair", tag="s_pair")
                nc.tensor.matmul(
                    s_pair[:, 0:w],
                    KT[0:DH, ks : ks + P],
                    QT[0:DH, qs : qs + w],
                    start=True, stop=True, tile_position=(0, 0),
                )
                nc.tensor.matmul(
                    s_pair[:, 512 : 512 + w],
                    KT[DH:P, ks : ks + P],
                    QT[DH:P, qs : qs + w],
                    start=True, stop=True, tile_position=(DH, 0),
                )
                e_pair = epool.tile([P, 2, 512], fp16, name="e_pair", tag="e_pair")
                nc.scalar.activation(
                    e_pair[:, :, :w],
                    s_pair.rearrange("p (h q) -> p h q", h=2)[:, :, :w],
                    Exp, bias=kbias_sb[:, gkt : gkt + 1], scale=DH**-0.5,
                )
                return e_pair

            def emit_pv(kt, e_pair):
                gkt = toff[j] + kt
                first, last = kt == 0, kt == nt - 1
                nc.tensor.matmul(
                    o_ps[0:DH, :w], V[:, gkt, 0:DH],
                    e_pair[:, 0, :w], start=first, stop=last,
                    tile_position=(0, 0), skip_group_check=True,
                )
                nc.tensor.matmul(
                    o_ps[DH:P, :w], V[:, gkt, DH:P],
                    e_pair[:, 1, :w], start=first, stop=last,
                    tile_position=(0, DH), skip_group_check=True,
                )
                nc.tensor.matmul(
                    d_ps[0:DH, :w], ones64, e_pair[:, 0, :w],
                    start=first, stop=last,
                    tile_position=(0, 0), skip_group_check=True,
                )
                nc.tensor.matmul(
                    d_ps[DH:P, :w], ones64, e_pair[:, 1, :w],
                    start=first, stop=last,
                    tile_position=(0, DH), skip_group_check=True,
                )

            pending = None
            for kt in range(nt):
                e_pair = emit_scores_exp(kt)
                if pending is not None:
                    emit_pv(*pending)
                pending = (kt, e_pair)
                if filler and iters_left[0] > 0:
                    k = -(-len(filler) // iters_left[0])
                    for _ in range(min(k, len(filler))):
                        filler.pop(0)()
                iters_left[0] -= 1
            emit_pv(*pending)
            rrep = epool.tile([P, 512], fp32, tag="rrep", bufs=2)
            nc.vector.reciprocal_approx_fast(out=rrep[:, :w], in_=d_ps[:, :w])
            nc.vector.tensor_tensor(
                attnT[:, qs : qs + w], o_ps[:, :w], rrep[:, :w], mult
            )

        # ---- choreographed emission ----
        # slot j's attention blocks absorb filler: QK/V of slot j+1 and
        # out-proj of already-finished chunks.
        chunks = []  # (slot, qs, w)
        for j, nt in enumerate(profile):
            qs = toff[j] * P
            for w in _qchunks(nt):
                chunks.append((j, qs, w))
                qs += w

        def slot_prep_units(j):
            us = []
            qs = toff[j] * P
            for w in _qchunks(profile[j]):
                us.extend(qk_units(QT, "wq", qs, w))
                us.extend(qk_units(KT, "wk", qs, w))
                qs += w
            for kt in range(profile[j]):
                us.extend(v_units(toff[j] + kt))
            return us

        # slot 0 prep emitted directly (nothing to overlap it with yet).
        # K and V first, then Q of chunk 0 — attention can start without the
        # later Q chunks, which drain as filler during chunk 0's k-loop.
        w0chunks = []
        qs = 0
        for w in _qchunks(profile[0]):
            w0chunks.append((qs, w))
            qs += w
        for qs, w in w0chunks:
            for u in qk_units(KT, "wk", qs, w):
                u()
        for kt in range(profile[0]):
            for u in v_units(kt):
                u()
        for u in qk_units(QT, "wq", w0chunks[0][0], w0chunks[0][1]):
            u()
        deferred_q0 = []
        for qs, w in w0chunks[1:]:
            deferred_q0.extend(qk_units(QT, "wq", qs, w))

        from itertools import groupby

        filler: list = list(deferred_q0)
        o_pending: list = []
        slot_chunks = [
            (j, [(qs, w) for _, qs, w in grp])
            for j, grp in groupby(chunks, key=lambda c: c[0])
        ]
        for j, chs in slot_chunks:
            # pace filler over the whole slot: finished slots' out-proj and
            # the next slot's QK/V prep drain evenly across all its iters
            if o_pending:
                filler.extend(o_units(o_pending))
                o_pending = []
            if j + 1 < len(profile):
                filler.extend(slot_prep_units(j + 1))
            iters_left = [profile[j] * len(chs)]
            for qs, w in chs:
                attn_chunk(j, qs, w, filler, iters_left)
            o_pending.extend(chs)
        if o_pending:
            filler.extend(o_units(o_pending, final=True))
        while filler:
            filler.pop(0)()

    nc.compile()
    return nc


def _get_program(profile: tuple[int, ...]):
    if profile not in _BUILD_CACHE:
        _BUILD_CACHE[profile] = _build_bass(profile)
    return _BUILD_CACHE[profile]


def kernel(x, seq_lens, Wq, Wk, Wv, Wo, bo) -> np.ndarray:
    from concourse.bass_utils import run_bass_kernel_spmd

    x = np.asarray(x, dtype=np.float32)
    seq_lens_np = np.asarray(seq_lens, dtype=np.int32)
    x16 = np.asarray(x, dtype=np.float16)

    nt = np.maximum(1, -(-seq_lens_np // P)).astype(np.int64)
    order = np.argsort(-seq_lens_np, kind="stable")
    pairs = [(int(order[2 * i]), int(order[2 * i + 1])) for i in range(N_SLOTS)]
    profile = tuple(int(max(nt[a], nt[b])) for a, b in pairs)
    groups = [[a for a, b in pairs], [b for a, b in pairs]]
    NT = sum(profile)
    C = NT * P
    toff = np.concatenate([[0], np.cumsum(profile)])[:-1]

    nc = _get_program(profile)

    # per-pair weight slices: [128, KC, 128] fp16, kc-major partition layout
    def slice_w_in(W, p):  # W[:, p*128:(p+1)*128] -> [128, 4, 128]
        ws = np.asarray(W, dtype=np.float16)[:, p * P : (p + 1) * P]
        return np.ascontiguousarray(ws.reshape(KC, P, P).transpose(1, 0, 2))

    def slice_wo(W, p):  # Wo[p*128:(p+1)*128, :] -> [128 d, 4 oc, 128 o]
        ws = np.asarray(W, dtype=np.float16)[p * P : (p + 1) * P, :]
        return np.ascontiguousarray(ws.reshape(P, KC, P).transpose(0, 1, 2))

    w_pair = [
        {
            "wq": slice_w_in(Wq, p),
            "wk": slice_w_in(Wk, p),
            "wv": slice_w_in(Wv, p),
            "wo": slice_wo(Wo, p),
        }
        for p in range(N_PAIRS)
    ]

    # per-group packed x^T and key-mask bias
    pos = np.arange(P, dtype=np.int32)
    g_xt, g_kb = [], []
    for g in range(N_GROUPS):
        xt = np.zeros((P, KC, C), dtype=np.float16)
        kb = np.full((P, NT), -60.0, dtype=np.float32)
        for j, s in enumerate(groups[g]):
            L = int(seq_lens_np[s])
            cs = int(toff[j]) * P
            t = x16[s, :L].T.reshape(KC, P, L).transpose(1, 0, 2)
            xt[:, :, cs : cs + L] = t
            for kt in range(profile[j]):
                valid = (kt * P + pos) < L
                kb[:, int(toff[j]) + kt] = np.where(valid, 0.0, -60.0)
        g_xt.append(xt)
        g_kb.append(kb)

    in_maps = []
    for c in range(N_CORES):
        g, p = c // N_PAIRS, c % N_PAIRS
        in_maps.append({"xt": g_xt[g], "kbias": g_kb[g], **w_pair[p]})

    trace = bool(int(os.environ.get("KERNEL_TRACE", "0")))
    res = run_bass_kernel_spmd(
        nc, in_maps, core_ids=list(range(N_CORES)), trace=trace
    )
    kernel.last_results = res

    bo32 = np.asarray(bo, dtype=np.float32)
    out = np.zeros((B, S, D), dtype=np.float32)
    for g in range(N_GROUPS):
        acc = np.zeros((P, KC, C), dtype=np.float32)
        for p in range(N_PAIRS):
            acc += res.results[g * N_PAIRS + p]["outp"].astype(np.float32)
        # acc[op, oc, q] -> out[q, oc*128+op]
        acc = acc.transpose(2, 1, 0).reshape(C, D)
        for j, s in enumerate(groups[g]):
            L = int(seq_lens_np[s])
            cs = int(toff[j]) * P
            out[s, :L] = acc[cs : cs + L] + bo32
    return out


# revision 26
# speedup vs baseline: 1.0249x; 1.0249x over previous
"""Ragged-sequence multi-head attention (B=16, S=1024, D=512, H=8, DH=64)
for 8 Trainium2 NeuronCores.

Strategy v2: shard by (head-pair x sequence-group). The 16 sequences are
sorted by length and paired (1st+2nd, 3rd+4th, ...); each pair defines one
"slot" whose static k-tile count is the max of the two -> a common slot
profile shared by both groups. Group A takes the first of each pair, group
B the second. Cores = 4 head-pairs x 2 groups; every core runs the SAME
SPMD program over its group's packed sequences, computing Q/K/V, ragged
attention (per-slot loop bounds = that slot's k-tile count), and a
TRANSPOSED partial output projection with its head-pair's Wo slice. The
host sums the 4 per-pair partials of each group and adds bo.

vs the 2-slot baseline this removes ~45% of the attention area (each slot
pays only its own length, not the global max) and ~35% of projection work.

Per-core pipeline:
  1. QT/KT [128=2x64 dims, C] via stationary Wq/Wk pair-slices (4 kc chunks)
  2. V [128 keys, kt, 128 dims] via stationary xT k-tiles, moving Wv slice
  3. per slot, per q-chunk, per k-tile:
       scoresT[k, q] = K^T q (row-packed head pair)
       expT = exp(0.125*scoresT + key_mask_bias)   (ACT)
       outT[d, q] += V^T expT, denom += 1^T expT   (col-packed pairs)
  4. attnT = outT * reciprocal(denom)              (DVE)
  5. partial^T[o, q] = Wo_chunk^T @ attnT (4 persistent 128-col stationaries)
"""

import math
import os

import numpy as np

B, S, D = 16, 1024, 512
H, DH = 8, 64
N_CORES = 8
P = 128
KC = D // P  # 4 contraction chunks of 128
N_PAIRS = 4  # head pairs
N_GROUPS = 2  # sequence groups
N_SLOTS = B // N_GROUPS  # 8 slots per group

_BUILD_CACHE: dict = {}


def _qchunks(nt: int) -> list[int]:
    """Split nt 128-col tiles into balanced chunks of <=4 tiles each."""
    n = -(-nt // 4)
    base, rem = divmod(nt, n)
    return [(base + (1 if i < rem else 0)) * P for i in range(n)]


def _build_bass(profile: tuple[int, ...]):
    """Build the SPMD Bass program for a slot profile (k-tile counts)."""
    from contextlib import ExitStack

    import concourse.bass as bass
    import concourse.mybir as mybir
    import concourse.tile as tile
    from concourse import bacc

    fp32 = mybir.dt.float32
    fp16 = mybir.dt.float16
    Exp = mybir.ActivationFunctionType.Exp
    mult = mybir.AluOpType.mult

    NT = sum(profile)  # total k-tiles
    C = NT * P  # packed columns
    toff = [0] * len(profile)  # slot -> first global k-tile
    for j in range(1, len(profile)):
        toff[j] = toff[j - 1] + profile[j - 1]

    nc = bacc.Bacc("TRN2", target_bir_lowering=False, debug=False)

    xt_d = nc.dram_tensor("xt", [P, KC, C], fp16, kind="ExternalInput").ap()
    w_d = {
        name: nc.dram_tensor(name, [P, KC, P], fp16, kind="ExternalInput").ap()
        for name in ("wq", "wk", "wv", "wo")
    }
    kbias_d = nc.dram_tensor("kbias", [P, NT], fp32, kind="ExternalInput").ap()
    outp_d = nc.dram_tensor("outp", [P, KC, C], fp16, kind="ExternalOutput").ap()

    with ExitStack() as ctx:
        tc = ctx.enter_context(tile.TileContext(nc))
        singles = ctx.enter_context(tc.tile_pool(name="singles", bufs=1))
        big = ctx.enter_context(tc.tile_pool(name="big", bufs=1))
        epool = ctx.enter_context(tc.tile_pool(name="epool", bufs=3))
        opool = ctx.enter_context(tc.tile_pool(name="opool", bufs=4))
        mmps = ctx.enter_context(tc.tile_pool(name="mmps", bufs=2, space="PSUM"))
        scps = ctx.enter_context(tc.tile_pool(name="scps", bufs=2, space="PSUM"))
        accps = ctx.enter_context(tc.tile_pool(name="accps", bufs=1, space="PSUM"))

        ones64 = singles.tile([P, DH], fp16)
        nc.vector.memset(ones64, 1.0)
        warm = singles.tile([P, 512], fp16)
        nc.vector.memset(warm, 0.0)

        w_sb = {
            name: singles.tile([P, KC, P], fp16, name=f"w_{name}")
            for name in ("wq", "wk", "wv", "wo")
        }
        kbias_sb = singles.tile([P, NT], fp32)

        # ---- input DMA: weights on sync queue, x^T chunks on gpsimd ----
        nc.sync.dma_start(out=w_sb["wq"], in_=w_d["wq"])
        nc.sync.dma_start(out=w_sb["wk"], in_=w_d["wk"])
        xT = big.tile([P, KC, C], fp16, name="xT", tag="xT")
        for cs in range(0, C, 512):
            w = min(512, C - cs)
            nc.gpsimd.dma_start(out=xT[:, :, cs : cs + w], in_=xt_d[:, :, cs : cs + w])
        nc.sync.dma_start(out=w_sb["wv"], in_=w_d["wv"])
        nc.sync.dma_start(out=kbias_sb, in_=kbias_d)
        nc.sync.dma_start(out=w_sb["wo"], in_=w_d["wo"])

        # ---- PE/ACT warm-up while input DMA streams (no data deps) ----
        act_dummy = singles.tile([P, 2], fp32)
        nc.scalar.activation(act_dummy[:, 0:1], warm[:, 0:1], Exp, scale=1.0)
        for i in range(30):
            wps = mmps.tile([P, 512], fp32, name="warm_ps", tag="mm")
            nc.tensor.matmul(wps[0:DH, :P], ones64, warm[:, :P], start=True, stop=True)

        QT = big.tile([P, C], fp16, name="QT", tag="QT")
        KT = big.tile([P, C], fp16, name="KT", tag="KT")
        V = big.tile([P, NT, P], fp16, name="V", tag="V")
        attnT = big.tile([P, C], fp16, name="attnT", tag="attnT")

        def qk_units(dst, wname, qs, w):
            """Project x cols [qs, qs+w) with the pair's Wq/Wk slice."""
            ps_box = []

            def mk_mm(kc):
                def emit():
                    if not ps_box:
                        ps_box.append(mmps.tile([P, 512], fp32, name="qk_ps", tag="mm"))
                    nc.tensor.matmul(
                        ps_box[0][:, :w],
                        w_sb[wname][:, kc, :],
                        xT[:, kc, qs : qs + w],
                        start=(kc == 0),
                        stop=(kc == KC - 1),
                    )
                return emit

            def fin():
                # half-partition copies so RAW deps match score matmul reads
                nc.vector.tensor_copy(out=dst[0:DH, qs : qs + w], in_=ps_box[0][0:DH, :w])
                nc.vector.tensor_copy(out=dst[DH:P, qs : qs + w], in_=ps_box[0][DH:P, :w])

            return [mk_mm(kc) for kc in range(KC)] + [fin]

        def v_units(gkt):
            """V tile for global k-tile gkt: [128 keys, 128 pair dims]."""
            ps_box = []

            def mk_mm(kc):
                def emit():
                    if not ps_box:
                        ps_box.append(mmps.tile([P, 512], fp32, name="v_ps", tag="mm"))
                    nc.tensor.matmul(
                        ps_box[0][:, :P],
                        xT[:, kc, gkt * P : (gkt + 1) * P],
                        w_sb["wv"][:, kc, :],
                        start=(kc == 0),
                        stop=(kc == KC - 1),
                    )
                return emit

            def fin():
                nc.vector.tensor_copy(out=V[:, gkt, :], in_=ps_box[0][:, :P])

            return [mk_mm(kc) for kc in range(KC)] + [fin]

        def o_units(chunk_list, final=False):
            """Transposed partial out-proj, oc-major over 1-2 q-chunks so
            each Wo chunk stationary is loaded once per batch. In the final
            drain (no exp work left) casts alternate scalar/vector."""
            units = []
            for oc in range(KC):
                for qs, w in chunk_list:
                    def mk(oc, qs, w):
                        ps_box = []

                        def emit_mm():
                            ps_box.append(
                                mmps.tile([P, 512], fp32, name="o_ps", tag="mm")
                            )
                            nc.tensor.matmul(
                                ps_box[0][:, :w],
                                w_sb["wo"][:, oc, :],
                                attnT[:, qs : qs + w],
                                start=True,
                                stop=True,
                            )

                        def emit_fin():
                            fout = opool.tile([P, 512], fp16, tag="fout")
                            if final and oc % 2 == 1:
                                nc.scalar.copy(out=fout[:, :w], in_=ps_box[0][:, :w])
                            else:
                                nc.vector.tensor_copy(
                                    out=fout[:, :w], in_=ps_box[0][:, :w]
                                )
                            nc.sync.dma_start(
                                out=outp_d[:, oc, qs : qs + w], in_=fout[:, :w]
                            )

                        return [emit_mm, emit_fin]

                    units.extend(mk(oc, qs, w))
            return units

        def attn_chunk(j, qs, w, filler, iters_left):
            """Attention for slot j, q cols [qs, qs+w), both heads of pair."""
            nt = profile[j]
            o_ps = accps.tile([P, 512], fp32, name="opv_ps", tag="opv_ps")
            d_ps = accps.tile([P, 512], fp32, name="d_ps", tag="d_ps")

            def emit_scores_exp(kt):
                gkt = toff[j] + kt
                ks = gkt * P
                s_pair = scps.tile([P, 1024], fp32, name="s_pair", tag="s_pair")
                nc.tensor.matmul(
                    s_pair[:, 0:w],
                    KT[0:DH, ks : ks + P],
                    QT[0:DH, qs : qs + w],
                    start=True, stop=True, tile_position=(0, 0),
                )
                nc.tensor.matmul(
                    s_pair[:, 512 : 512 + w],
                    KT[DH:P, ks : ks + P],
                    QT[DH:P, qs : qs + w],
                    start=True, stop=True, tile_position=(DH, 0),
                )
                e_pair = epool.tile([P, 2, 512], fp16, name="e_pair", tag="e_pair")
                nc.scalar.activation(
                    e_pair[:, :, :w],
                    s_pair.rearrange("p (h q) -> p h q", h=2)[:, :, :w],
                    Exp, bias=kbias_sb[:, gkt : gkt + 1], scale=DH**-0.5,
                )
                return e_pair

            def emit_pv(kt, e_pair):
                gkt = toff[j] + kt
                first, last = kt == 0, kt == nt - 1
                nc.tensor.matmul(
                    o_ps[0:DH, :w], V[:, gkt, 0:DH],
                    e_pair[:, 0, :w], start=first, stop=last,
                    tile_position=(0, 0), skip_group_check=True,
                )
                nc.tensor.matmul(
                    o_ps[DH:P, :w], V[:, gkt, DH:P],
                    e_pair[:, 1, :w], start=first, stop=last,
                    tile_position=(0, DH), skip_group_check=True,
                )
                nc.tensor.matmul(
                    d_ps[0:DH, :w], ones64, e_pair[:, 0, :w],
                    start=first, stop=last,
                    tile_position=(0, 0), skip_group_check=True,
                )
                nc.tensor.matmul(
                    d_ps[DH:P, :w], ones64, e_pair[:, 1, :w],
                    start=first, stop=last,
                    tile_position=(0, DH), skip_group_check=True,
                )

            pending = None
            for kt in range(nt):
                e_pair = emit_scores_exp(kt)
                if pending is not None:
                    emit_pv(*pending)
                pending = (kt, e_pair)
                if filler and iters_left[0] > 0:
                    k = -(-len(filler) // iters_left[0])
                    for _ in range(min(k, len(filler))):
                        filler.pop(0)()
                iters_left[0] -= 1
            emit_pv(*pending)
            rrep = epool.tile([P, 512], fp32, tag="rrep", bufs=2)
            nc.vector.reciprocal_approx_fast(out=rrep[:, :w], in_=d_ps[:, :w])
            nc.vector.tensor_tensor(
                attnT[:, qs : qs + w], o_ps[:, :w], rrep[:, :w], mult
            )

        # ---- choreographed emission ----
        # slot j's attention blocks absorb filler: QK/V of slot j+1 and
        # out-proj of already-finished chunks.
        chunks = []  # (slot, qs, w)
        for j, nt in enumerate(profile):
            qs = toff[j] * P
            for w in _qchunks(nt):
                chunks.append((j, qs, w))
                qs += w

        def slot_prep_units(j):
            us = []
            qs = toff[j] * P
            for w in _qchunks(profile[j]):
                us.extend(qk_units(QT, "wq", qs, w))
                us.extend(qk_units(KT, "wk", qs, w))
                qs += w
            for kt in range(profile[j]):
                us.extend(v_units(toff[j] + kt))
            return us

        # slot 0 prep emitted directly (nothing to overlap it with yet)
        for u in slot_prep_units(0):
            u()

        from itertools import groupby

        filler: list = []
        o_pending: list = []
        slot_chunks = [
            (j, [(qs, w) for _, qs, w in grp])
            for j, grp in groupby(chunks, key=lambda c: c[0])
        ]
        for j, chs in slot_chunks:
            # pace filler over the whole slot: finished slots' out-proj and
            # the next slot's QK/V prep drain evenly across all its iters
            if o_pending:
                filler.extend(o_units(o_pending))
                o_pending = []
            if j + 1 < len(profile):
                filler.extend(slot_prep_units(j + 1))
            iters_left = [profile[j] * len(chs)]
            for qs, w in chs:
                attn_chunk(j, qs, w, filler, iters_left)
            o_pending.extend(chs)
        if o_pending:
            filler.extend(o_units(o_pending))
        while filler:
            filler.pop(0)()

    nc.compile()
    return nc


def _get_program(profile: tuple[int, ...]):
    if profile not in _BUILD_CACHE:
        _BUILD_CACHE[profile] = _build_bass(profile)
    return _BUILD_CACHE[profile]


def kernel(x, seq_lens, Wq, Wk, Wv, Wo, bo) -> np.ndarray:
    from concourse.bass_utils import run_bass_kernel_spmd

    x = np.asarray(x, dtype=np.float32)
    seq_lens_np = np.asarray(seq_lens, dtype=np.int32)
    x16 = np.asarray(x, dtype=np.float16)

    nt = np.maximum(1, -(-seq_lens_np // P)).astype(np.int64)
    order = np.argsort(-seq_lens_np, kind="stable")
    pairs = [(int(order[2 * i]), int(order[2 * i + 1])) for i in range(N_SLOTS)]
    profile = tuple(int(max(nt[a], nt[b])) for a, b in pairs)
    groups = [[a for a, b in pairs], [b for a, b in pairs]]
    NT = sum(profile)
    C = NT * P
    toff = np.concatenate([[0], np.cumsum(profile)])[:-1]

    nc = _get_program(profile)

    # per-pair weight slices: [128, KC, 128] fp16, kc-major partition layout
    def slice_w_in(W, p):  # W[:, p*128:(p+1)*128] -> [128, 4, 128]
        ws = np.asarray(W, dtype=np.float16)[:, p * P : (p + 1) * P]
        return np.ascontiguousarray(ws.reshape(KC, P, P).transpose(1, 0, 2))

    def slice_wo(W, p):  # Wo[p*128:(p+1)*128, :] -> [128 d, 4 oc, 128 o]
        ws = np.asarray(W, dtype=np.float16)[p * P : (p + 1) * P, :]
        return np.ascontiguousarray(ws.reshape(P, KC, P).transpose(0, 1, 2))

    w_pair = [
        {
            "wq": slice_w_in(Wq, p),
            "wk": slice_w_in(Wk, p),
            "wv": slice_w_in(Wv, p),
            "wo": slice_wo(Wo, p),
        }
        for p in range(N_PAIRS)
    ]

    # per-group packed x^T and key-mask bias
    pos = np.arange(P, dtype=np.int32)
    g_xt, g_kb = [], []
    for g in range(N_GROUPS):
        xt = np.zeros((P, KC, C), dtype=np.float16)
        kb = np.full((P, NT), -60.0, dtype=np.float32)
        for j, s in enumerate(groups[g]):
            L = int(seq_lens_np[s])
            cs = int(toff[j]) * P
            t = x16[s, :L].T.reshape(KC, P, L).transpose(1, 0, 2)
            xt[:, :, cs : cs + L] = t
            for kt in range(profile[j]):
                valid = (kt * P + pos) < L
                kb[:, int(toff[j]) + kt] = np.where(valid, 0.0, -60.0)
        g_xt.append(xt)
        g_kb.append(kb)

    in_maps = []
    for c in range(N_CORES):
        g, p = c // N_PAIRS, c % N_PAIRS
        in_maps.append({"xt": g_xt[g], "kbias": g_kb[g], **w_pair[p]})

    trace = bool(int(os.environ.get("KERNEL_TRACE", "0")))
    res = run_bass_kernel_spmd(
        nc, in_maps, core_ids=list(range(N_CORES)), trace=trace
    )
    kernel.last_results = res

    bo32 = np.asarray(bo, dtype=np.float32)
    out = np.zeros((B, S, D), dtype=np.float32)
    for g in range(N_GROUPS):
        acc = np.zeros((P, KC, C), dtype=np.float32)
        for p in range(N_PAIRS):
            acc += res.results[g * N_PAIRS + p]["outp"].astype(np.float32)
        # acc[op, oc, q] -> out[q, oc*128+op]
        acc = acc.transpose(2, 1, 0).reshape(C, D)
        for j, s in enumerate(groups[g]):
            L = int(seq_lens_np[s])
            cs = int(toff[j]) * P
            out[s, :L] = acc[cs : cs + L] + bo32
    return out
